# revision 26
# baseline (speedup 1.0000x reference)
"""FLAttention Bass/Tile kernel for Trainium2, batch-sharded over 8 NeuronCores.

Math per (head h, batch row b), with x_b = x[b, :] (D=512):
    q_j = a_q x_j + b_q ; k_i = a_k x_i + b_k ; v_j = a_v x_j + b_v
    u[i,j] = |q_j - k_i| + eps     (here: max(|z|, eps), equal to fp32 noise)
    w[i,j] = softmax_j(1/u[i,j])   (stabilized by the row max m = 1/umin)
    att[i] = sum_j w[i,j] v_j / sqrt(D)
    out = x + sum_h att_h

Per-core plan (8 batch rows x 8 heads = 64 pairs, D^2 = 512x512 each):
  [i,j] orientation (i on partitions, 4 chunks of 128; j on free, 512):
  1. u-pass: u = max(|a_q*xb + kk|, eps) with fused row-min accumulator.
     One fused custom DVE op per chunk, or ACT Abs + Pool eps/min assist
     (engine split is a tuning knob).
  2. One big NEG_RECIP_FAST over [128, 4*512+4]: negr = -1/u; the umin
     tail gives negm = -m with the same approximation, so the dominant
     softmax element cancels exactly.
  3. msub: s' = r - m as fp16 (overflows to -inf for hopeless elements,
     which exp correctly turns into 0). Split across ACT (Identity with
     scale=-1, bias=negm), DVE and Pool tensor_scalar.
  4. ONE DMA-XBAR transpose of s' [128, 2048] fp16 -> g-major transposed
     layout (16 blocks of [128,128]); runs on the otherwise idle DMA
     engines, j moves onto partitions.
  5. ONE big ACT Exp -> e (bf16).
  6. PE matmuls (4 per pair): lhsT = [ones | v] bf16 [128, 2],
     rhs = e j-chunk (strided 3D AP), accumulating S and SV rows into a
     PSUM bank; 3 pairs share a bank (partition slots 0/32/64).
  7. Whole-bank drain every 3 pairs; batched epilogue: PE-transpose the
     stats, reciprocal+mult+head-sum, out = x + att/sqrt(D).
"""

import os

import numpy as np

import concourse.bacc as bacc
import concourse.bass as bass
import concourse.dve_ops as dve_ops
import concourse.mybir as mybir
import concourse.tile as tile
from concourse.bass_utils import run_bass_kernel_spmd
from concourse.dve_spec import (
    AluOp,
    Bin,
    C0,
    C1,
    C2,
    One,
    Spec,
    Src0,
    Zero,
    lower,
    maxx,
    minn,
)
from concourse.dve_uop import DveOpSpec
from concourse.masks import make_identity


def _register(name, spec):
    for op in dve_ops.OPS:
        if op.name == name:
            return op
    op = dve_ops.DveOp(name, spec, subdim=False, uops_sha={})
    dve_ops.OPS.append(op)
    dve_ops.CUSTOM_DVE_SPECS[name] = op.spec
    dve_ops._SUB_OPCODE_FOR_NAME[name] = max(dve_ops._SUB_OPCODE_FOR_NAME.values()) + 1
    shas = {}
    for ver in ("v3", "v4"):
        ospec = DveOpSpec(
            name=name,
            opcode=dve_ops.get_dve_sub_opcode(name),
            uops=lower(op.spec, ver=ver),
            rd1_en=False,
        )
        shas[ver] = ospec.sha(ver)
    object.__setattr__(op, "uops_sha", shas)
    return op


# u = max(|x*a + k|, eps) with fused row-min accumulator (init 1.0 -- a
# stabilizer clamp m <= 1 is still overflow-safe: then every r <= 1).
_z = Src0 * C0 + C1
UPASS_OP = _register(
    "UPASS_ANT",
    Spec(body=maxx(maxx(_z, Zero - _z), C2), accum=minn, accum_init=One),
)

# negated fast reciprocal (same stages as RECIPROCAL_APPROX_FAST with the
# final subtract flipped): out = -1/in at ~51 ULP.
_not_x = Bin(AluOp.BITWISE_NOT, Src0, Src0)
_y0 = _not_x * C0
_y1 = _y0 * (C1 - Src0 * _y0)


def _ref_neg_fast(in0, in1, c0, c1, c2):
    not_x = (~in0.view(np.int32)).view(np.float32)
    y0 = (not_x * np.float32(c0)).astype(np.float32)
    y1 = (y0 * (np.float32(c1) - in0 * y0)).astype(np.float32)
    return (y1 * (in0 * y1 - np.float32(c2))).astype(np.float32)


# negr = -1/u (negated RECIPROCAL_APPROX_FAST, ~51 ULP). Applied to the
# umin tail too, so -m uses the same approximation and the top softmax
# element cancels exactly.
NEG_FAST_OP = _register(
    "NEG_RECIP_FAST_ANT",
    Spec(body=_y1 * (Src0 * _y1 - C2), reference=_ref_neg_fast),
)

B, D, H = 64, 512, 8
NCORES = 8
RB = B // NCORES
NCH = D // 128
EPS = 1e-8
INV_SQRT_D = float(np.float32(1.0) / np.sqrt(np.float32(D)))
NPAIR = RB * H  # 64

F32 = mybir.dt.float32
BF16 = mybir.dt.bfloat16
FP16 = mybir.dt.float16
AF = mybir.ActivationFunctionType
OP = mybir.AluOpType
AX = mybir.AxisListType

# ---- tuning knobs -------------------------------------------------------
# per-chunk engine for the u-pass: 'd' = fused custom DVE op,
# 'a' = ACT Abs + eps/min on Pool, 'A' = ACT Abs + eps/min on DVE,
# 'p' = two Pool tensor_scalars.
U_SPLIT = os.environ.get("KBENCH_U", "ddaa")
# per-chunk engine for msub (s' = r - m): 'a' ACT, 'd' DVE, 'p' Pool
MSUB_SPLIT = os.environ.get("KBENCH_MSUB", "appd")
# stats drain engine rotation: string over {'d','a','p'}, cycled per group
DRAIN_ENG = os.environ.get("KBENCH_DRAIN", "da")
LOOKAHEAD = int(os.environ.get("KBENCH_LOOKAHEAD", "2"))
GSIZE = 3  # pairs per PSUM stats bank (partition slots 0/32/64)
# transpose issue queues, cycled per pair: 's' = SP, 'a' = ACT
TQ = os.environ.get("KBENCH_TQ", "sa")
NOTRANS = os.environ.get("KBENCH_NOTRANS", "0") == "1"  # timing-only ablation
NOMM = os.environ.get("KBENCH_NOMM", "0") == "1"  # timing-only ablation
NOEXP = os.environ.get("KBENCH_NOEXP", "0") == "1"  # timing-only ablation
NOFRONT = os.environ.get("KBENCH_NOFRONT", "0") == "1"  # timing-only ablation
TB = int(os.environ.get("KBENCH_TB", "4"))  # pairs per batched transpose


def build_nc():
    nc = bacc.Bacc(
        "TRN2",
        target_bir_lowering=False,
        debug=False,
        num_devices=NCORES,
    )
    x_d = nc.dram_tensor("x_shard", [RB, D], F32, kind="ExternalInput")
    al_d = nc.dram_tensor("alphas", [H, 3], F32, kind="ExternalInput")
    be_d = nc.dram_tensor("betas", [H, 3], F32, kind="ExternalInput")
    out_d = nc.dram_tensor("out_shard", [RB, D], F32, kind="ExternalOutput")

    n_groups = (NPAIR + GSIZE - 1) // GSIZE

    with tile.TileContext(nc) as tc:
        with (
            tc.tile_pool(name="const", bufs=1) as cpool,
            tc.tile_pool(name="ubig", bufs=2) as upool,
            tc.tile_pool(name="rbig", bufs=2) as rpool,
            tc.tile_pool(name="sbig", bufs=3) as spool,
            tc.tile_pool(name="tbatch", bufs=2) as tpool,
            tc.tile_pool(name="work", bufs=4) as wpool,
            tc.tile_pool(name="psum", bufs=3, space="PSUM") as ppool,
            tc.tile_pool(name="epips", bufs=2, space="PSUM") as epool,
        ):
            # ---------------- prologue ----------------
            ones = cpool.tile([1, 128], F32, tag="ones")
            nc.vector.memset(ones[:], 1.0)

            x_flat = cpool.tile([1, RB * D], F32, tag="xflat")
            nc.sync.dma_start(
                x_flat[:], x_d.ap().rearrange("(o b) d -> o (b d)", o=1)
            )
            ab_row = cpool.tile([1, 48], F32, tag="abrow")
            nc.sync.dma_start(
                ab_row[:, 0:24], al_d.ap().rearrange("(o h) t -> o (h t)", o=1)
            )
            nc.sync.dma_start(
                ab_row[:, 24:48], be_d.ap().rearrange("(o h) t -> o (h t)", o=1)
            )
            x_part = cpool.tile([128, RB * NCH], F32, tag="xpart")
            nc.sync.dma_start(
                x_part[:], x_d.ap().rearrange("b (c p) -> p (b c)", p=128)
            )

            with tc.tile_pool(name="propsum", bufs=1, space="PSUM") as prop:
                ab_psum = prop.tile([128, 48], F32, tag="abp")
                nc.tensor.matmul(ab_psum[:], ones[:], ab_row[:])
                AB = cpool.tile([128, 48], F32, tag="ab")
                nc.vector.tensor_copy(AB[:], ab_psum[:])

                xb_sbuf = []
                for b in range(RB):
                    pt = prop.tile([128, D], F32, tag=f"xb{b % 2}")
                    nc.tensor.matmul(
                        pt[:], ones[:], x_flat[:, b * D : (b + 1) * D]
                    )
                    st = cpool.tile([128, D], F32, tag=f"xs{b}")
                    nc.vector.tensor_copy(st[:], pt[:])
                    xb_sbuf.append(st)

            ab3 = AB[:, 0:24].rearrange("p (h t) -> p h t", t=3)
            bb3 = AB[:, 24:48].rearrange("p (h t) -> p h t", t=3)
            negak = cpool.tile([128, H], F32, tag="negak")
            nc.vector.tensor_scalar_mul(negak[:], ab3[:, :, 1], -1.0)
            bqk = cpool.tile([128, H], F32, tag="bqk")
            nc.vector.tensor_sub(bqk[:], bb3[:, :, 0], bb3[:, :, 1])

            # KK[:, (h, b, c)] = -(a_k x_i + b_k - b_q)
            KK = cpool.tile([128, H * RB * NCH], F32, tag="kk")
            for h in range(H):
                nc.vector.tensor_scalar(
                    KK[:, h * RB * NCH : (h + 1) * RB * NCH],
                    x_part[:],
                    negak[:, h : h + 1],
                    bqk[:, h : h + 1],
                    op0=OP.mult,
                    op1=OP.add,
                )

            # V2: interleaved [ones | v] columns, bf16.
            # col 2t = 1, col 2t+1 = a_v x + b_v, t = (h*RB + b)*NCH + c
            V2 = cpool.tile([128, 2 * H * RB * NCH], BF16, tag="v2")
            v2v = V2[:].rearrange("p (t two) -> p t two", two=2)
            nc.vector.memset(v2v[:, :, 0:1], 1.0)
            for h in range(H):
                nc.vector.tensor_scalar(
                    v2v[:, h * RB * NCH : (h + 1) * RB * NCH, 1:2].rearrange(
                        "p t o -> p (t o)"
                    ),
                    x_part[:],
                    ab3[:, h : h + 1, 2:3].rearrange("p h t -> p (h t)"),
                    bb3[:, h : h + 1, 2:3].rearrange("p h t -> p (h t)"),
                    op0=OP.mult,
                    op1=OP.add,
                )

            ident = cpool.tile([128, 128], F32, tag="ident")
            make_identity(nc, ident[:])

            stats_sb = cpool.tile([128, n_groups * D], F32, tag="statsb")

            rc = dve_ops.RECIP_APPROX_FAST_CONSTS

            # ---------------- main loop ----------------
            pairs = [(b, h) for b in range(RB) for h in range(H)]
            state = {}

            def emit_front(p):
                b, h = pairs[p]
                if NOFRONT:
                    negr0 = rpool.tile([128, NCH * D + NCH], F32, tag="r")
                    nc.vector.memset(negr0[:, 0:1], -1.0)
                    state[p] = negr0
                    return
                u_big = upool.tile([128, NCH * D + NCH], F32, tag="u")
                for c, eng in enumerate(U_SPLIT):
                    kcol = h * RB * NCH + b * NCH + c
                    uch = u_big[:, c * D : (c + 1) * D]
                    acc = u_big[:, NCH * D + c : NCH * D + c + 1]
                    if eng == "d":
                        nc.vector._custom_dve(
                            UPASS_OP,
                            out=uch,
                            in0=xb_sbuf[b][:],
                            s0=AB[:, 3 * h : 3 * h + 1],
                            s1=KK[:, kcol : kcol + 1],
                            imm2=EPS,
                            accum_out=acc,
                        )
                    else:  # 'A': ACT Abs then eps-max + min-accum on DVE
                        t_t = wpool.tile([128, D], F32, tag="t")
                        nc.scalar.activation(
                            t_t[:],
                            xb_sbuf[b][:],
                            AF.Abs,
                            bias=KK[:, kcol : kcol + 1],
                            scale=AB[:, 3 * h : 3 * h + 1],
                        )
                        nc.vector.tensor_scalar(
                            uch,
                            t_t[:],
                            EPS,
                            None,
                            op0=OP.max,
                            op1=OP.min,
                            accum_out=acc,
                        )
                # one big negr = -1/u including the umin tail -> -m
                negr = rpool.tile([128, NCH * D + NCH], F32, tag="r")
                nc.vector._custom_dve(
                    NEG_FAST_OP,
                    out=negr[:],
                    in0=u_big[:],
                    s0=rc["s0"],
                    s1=rc["s1"],
                    imm2=rc["imm2"],
                )
                state[p] = negr

            def emit_msub(p):
                negr = state.pop(p)
                t, tb = p % TB, p // TB
                if t == 0:
                    s16b = tpool.tile([128, TB * NCH * D], FP16, tag="s")
                    state[("s16", tb)] = s16b
                s16 = state[("s16", tb)][:, t * NCH * D : (t + 1) * NCH * D]
                for c, eng in enumerate(MSUB_SPLIT):
                    sch = s16[:, c * D : (c + 1) * D]
                    rch = negr[:, c * D : (c + 1) * D]
                    negm = negr[:, NCH * D + c : NCH * D + c + 1]
                    if eng == "a":
                        # s' = (-1)*negr + negm = r - m
                        nc.scalar.activation(
                            sch, rch, AF.Identity, bias=negm, scale=-1.0
                        )
                    else:
                        v_eng = nc.vector if eng == "d" else nc.gpsimd
                        # s' = (negr - negm) * -1 = r - m
                        v_eng.tensor_scalar(
                            sch,
                            rch,
                            negm,
                            -1.0,
                            op0=OP.subtract,
                            op1=OP.mult,
                        )

            def emit_trans(tb):
                # one batched XBAR transpose for TB pairs:
                # sT[p, g*128+q] = s16[q, g*128+p]; pair t's data stays in
                # slice [t*2048, (t+1)*2048) in the same g-major layout.
                s16b = state.pop(("s16", tb))
                sTb = tpool.tile([128, TB * NCH * D], FP16, tag="sT")
                if NOTRANS:
                    nc.vector.tensor_copy(sTb[:, 0:1], s16b[:, 0:1])
                else:
                    q_eng = nc.sync if TQ[tb % len(TQ)] == "s" else nc.scalar
                    q_eng.dma_start_transpose(
                        sTb[:].rearrange("p (g q) -> p g q", q=128), s16b[:]
                    )
                state[("sT", tb)] = sTb

            def emit_exp(p):
                t, tb = p % TB, p // TB
                sT = state[("sT", tb)][:, t * NCH * D : (t + 1) * NCH * D]
                if t == TB - 1:
                    state.pop(("sT", tb))
                e_big = spool.tile([128, NCH * D], BF16, tag="e")
                if NOEXP:
                    nc.vector.tensor_copy(e_big[:, 0:1], sT[:, 0:1])
                else:
                    nc.scalar.activation(
                        e_big[:], sT[:], AF.Exp, bias=0.0, scale=1.0
                    )
                state[p] = e_big

            def emit_back(p):
                b, h = pairs[p]
                e_big = state.pop(p)
                g, slot = divmod(p, GSIZE)
                if slot == 0:
                    bank_t = ppool.tile([128, D], F32, tag="bank")
                    state[("bank", g)] = bank_t
                bank = state[("bank", g)]
                ev = e_big[:].rearrange("p (ci cj q) -> p ci cj q", ci=NCH, cj=NCH)
                for cj in range(NCH):
                    if NOMM:
                        continue
                    tcol = (h * RB + b) * NCH + cj
                    nc.tensor.matmul(
                        bank[32 * slot : 32 * slot + 2, :],
                        V2[:, 2 * tcol : 2 * tcol + 2],
                        ev[:, :, cj : cj + 1, :].rearrange("p a b q -> p (a b) q"),
                        start=(cj == 0),
                        stop=(cj == NCH - 1),
                    )
                if slot == GSIZE - 1 or p == NPAIR - 1:
                    bank = state.pop(("bank", g))
                    eng = DRAIN_ENG[g % len(DRAIN_ENG)]
                    dst = stats_sb[:, g * D : (g + 1) * D]
                    if eng == "a":
                        nc.scalar.activation(dst, bank[:], AF.Copy)
                    elif eng == "p":
                        nc.gpsimd.tensor_copy(dst, bank[:])
                    else:
                        nc.vector.tensor_copy(dst, bank[:])

            def emit_main():
                # batch-level software pipeline: front+msub of batch k+1
                # overlaps transpose/exp/matmul of batch k.
                n = len(pairs)
                nb = n // TB
                for k in range(nb + 1):
                    if k < nb:
                        for t in range(TB):
                            emit_front(k * TB + t)
                            emit_msub(k * TB + t)
                        emit_trans(k)
                    if k >= 1:
                        for t in range(TB):
                            emit_exp((k - 1) * TB + t)
                            emit_back((k - 1) * TB + t)

            repeat = int(os.environ.get("KBENCH_REPEAT", "1"))
            if repeat > 1:
                with tc.For_i(0, repeat, 1):
                    emit_main()
            else:
                emit_main()

            # ---------------- epilogue ----------------
            # zT[i_low, pair*NCH + c] = SV/S for that pair and i-chunk
            zT = cpool.tile([128, NPAIR * NCH], F32, tag="zT")
            for g in range(n_groups):
                npair_g = min(GSIZE, NPAIR - g * GSIZE)
                for c in range(NCH):
                    T = epool.tile([128, 128], F32, tag="T")
                    nc.tensor.transpose(
                        T[:], stats_sb[:, g * D + c * 128 : g * D + (c + 1) * 128],
                        ident[:],
                    )
                    tv = T[:].rearrange("p (s r) -> p s r", r=32)
                    s_cols = tv[:, :npair_g, 0:1].rearrange("p s r -> p (s r)")
                    sv_cols = tv[:, :npair_g, 1:2].rearrange("p s r -> p (s r)")
                    sinv = wpool.tile([128, GSIZE], F32, tag="sinv")
                    nc.vector._custom_dve(
                        dve_ops.RECIPROCAL_APPROX_FAST,
                        out=sinv[:, :npair_g],
                        in0=s_cols,
                        s0=rc["s0"],
                        s1=rc["s1"],
                        imm2=rc["imm2"],
                    )
                    zv = zT[:].rearrange("p (q c) -> p q c", c=NCH)
                    nc.vector.tensor_tensor(
                        zv[:, g * GSIZE : g * GSIZE + npair_g, c : c + 1].rearrange(
                            "p q c -> p (q c)"
                        ),
                        sv_cols,
                        sinv[:, :npair_g],
                        op=OP.mult,
                    )
            # att[i_low, (b, c)] = sum_h zT[:, ((b*H+h))*NCH + c]
            att = cpool.tile([128, RB * NCH], F32, tag="att")
            zv4 = zT[:].rearrange("p (b h c) -> p b h c", h=H, c=NCH)
            for b in range(RB):
                nc.vector.tensor_reduce(
                    att[:, b * NCH : (b + 1) * NCH],
                    zv4[:, b : b + 1, :, :].rearrange("p b h c -> p (b c) h"),
                    axis=AX.X,
                    op=OP.add,
                )
            out_c = cpool.tile([128, RB * NCH], F32, tag="outc")
            nc.vector.scalar_tensor_tensor(
                out=out_c[:],
                in0=att[:],
                scalar=INV_SQRT_D,
                in1=x_part[:],
                op0=OP.mult,
                op1=OP.add,
            )
            nc.sync.dma_start(
                out_d.ap().rearrange("b (c p) -> p b c", p=128),
                out_c[:].rearrange("p (b c) -> p b c", c=NCH),
            )

    nc.compile()
    return nc


_NC_CACHE = None


def get_nc():
    global _NC_CACHE
    if _NC_CACHE is None:
        _NC_CACHE = build_nc()
    return _NC_CACHE


def kernel(x: np.ndarray, alphas: np.ndarray, betas: np.ndarray) -> np.ndarray:
    x = np.ascontiguousarray(x, dtype=np.float32)
    alphas = np.ascontiguousarray(alphas, dtype=np.float32)
    betas = np.ascontiguousarray(betas, dtype=np.float32)

    nc = get_nc()
    in_maps = [
        {
            "x_shard": x[c * RB : (c + 1) * RB],
            "alphas": alphas,
            "betas": betas,
        }
        for c in range(NCORES)
    ]
    res = run_bass_kernel_spmd(nc, in_maps, core_ids=list(range(NCORES)))
    out = np.concatenate([res.results[c]["out_shard"] for c in range(NCORES)], axis=0)
    return out


if __name__ == "__main__":
    rng = np.random.default_rng(0)
    x = rng.standard_normal((B, D), dtype=np.float32)
    al = rng.random((H, 3), dtype=np.float32)
    be = rng.random((H, 3), dtype=np.float32)
    out = kernel(x=x, alphas=al, betas=be)
    print("out", out.shape, out.dtype, float(np.abs(out).max()))


# revision 27
# speedup vs baseline: 3.2640x; 3.2640x over previous
"""FLAttention Bass/Tile kernel for Trainium2, batch-sharded over 8 NeuronCores.

Math per (head h, batch row b), with x_b = x[b, :] (D=512):
    q_j = a_q x_j + b_q ; k_i = a_k x_i + b_k ; v_j = a_v x_j + b_v
    u[i,j] = |q_j - k_i| + eps
    w[i,j] = softmax_j(1/u[i,j])                 (stabilized by row max)
    att[i] = sum_j w[i,j] v_j / sqrt(D)
    out = x + sum_h att_h

Per-core implementation (8 batch rows per core, all 8 heads):
  i (output row) on SBUF partitions (4 chunks of 128), j on free dim (512).
  - PE broadcasts x_b across partitions (outer product with ones) -> PSUM.
  - ACT computes t = Abs(a_q * X - kk_i)  (per-partition scale and bias APs).
  - DVE tensor_scalar: u = t + eps with fused row-min accumulator.
  - DVE reciprocal_approx_fast: r = 1/u  (~51 ULP).  The softmax stabilizer
    m = 1/u_min is computed with the SAME approximation so the dominant
    element cancels exactly and exp never overflows.
  - ACT Exp with per-partition bias -m and fused row-sum accumulator -> e, S.
  - GPSIMD multiplies e by v = a_v x + b_v; DVE accumulates SV = sum_j e*v
    with a 2x-mode tensor_scalar row-sum (engine-balanced three-way split).
  - Batched epilogue: att = SV/S summed over heads, out = x + att/sqrt(D).
  The pair loop is software-pipelined (lookahead 2) so the ACT->POOL->DVE
  round trip of one (head, row) pair overlaps the next pair's front half.
"""

import os

import numpy as np

import concourse.bacc as bacc
import concourse.bass as bass
import concourse.dve_ops as dve_ops
import concourse.mybir as mybir
import concourse.tile as tile
from concourse.bass_utils import run_bass_kernel_spmd
from concourse.dve_spec import AluOp, Bin, C0, C1, C2, Spec, Src0, lower
from concourse.dve_uop import DveOpSpec


def _register_neg_fast():
    """NEG_RECIP_FAST_ANT: out = -1/in at ~51 ULP (negated
    RECIPROCAL_APPROX_FAST -- same stages, final subtract flipped, so the
    row-min of the output is exactly -(1/u_min), the exp stabilizer bias)."""
    name = "NEG_RECIP_FAST_ANT"
    for op in dve_ops.OPS:
        if op.name == name:
            return op
    _not_x = Bin(AluOp.BITWISE_NOT, Src0, Src0)
    _y0 = _not_x * C0
    _y1 = _y0 * (C1 - Src0 * _y0)
    body = _y1 * (Src0 * _y1 - C2)

    def _ref(in0, in1, c0, c1, c2):
        not_x = (~in0.view(np.int32)).view(np.float32)
        y0 = (not_x * np.float32(c0)).astype(np.float32)
        y1 = (y0 * (np.float32(c1) - in0 * y0)).astype(np.float32)
        return (y1 * (in0 * y1 - np.float32(c2))).astype(np.float32)

    op = dve_ops.DveOp(
        name, Spec(body=body, reference=_ref), subdim=False, uops_sha={}
    )
    dve_ops.OPS.append(op)
    dve_ops.CUSTOM_DVE_SPECS[name] = op.spec
    dve_ops._SUB_OPCODE_FOR_NAME[name] = (
        max(dve_ops._SUB_OPCODE_FOR_NAME.values()) + 1
    )
    shas = {}
    for ver in ("v3", "v4"):
        ospec = DveOpSpec(
            name=name,
            opcode=dve_ops.get_dve_sub_opcode(name),
            uops=lower(op.spec, ver=ver),
            rd1_en=False,
        )
        shas[ver] = ospec.sha(ver)
    object.__setattr__(op, "uops_sha", shas)
    return op


NEG_FAST_OP = _register_neg_fast()

# tuning knobs (read once at build time)
EV_GPSIMD = os.environ.get("KBENCH_EV_GPSIMD", "1") == "1"
V_GPSIMD = os.environ.get("KBENCH_V_GPSIMD", "1") == "1"
BF16_EV = os.environ.get("KBENCH_BF16_EV", "0") == "1"
MERGED_MULT = os.environ.get("KBENCH_MERGED_MULT", "0") == "1"
RSUM = os.environ.get("KBENCH_RSUM", "0") == "1"
ABS_DVE = int(os.environ.get("KBENCH_ABS_DVE", "0"))  # chunks 0..n-1 on DVE
# every Nth chunk's e*v falls back to DVE affine_mul_reduce (0 = never):
AMR_EVERY = int(os.environ.get("KBENCH_AMR_EVERY", "0"))
# timing-only stage skips: comma list from {sv,exp,fast,tsu,abs}
SKIP = set(filter(None, os.environ.get("KBENCH_SKIP", "").split(",")))

B, D, H = 64, 512, 8
NCORES = 8
RB = B // NCORES          # batch rows per core
NCH = D // 128            # 4 partition chunks of the i dimension
EPS = 1e-8
INV_SQRT_D = float(np.float32(1.0) / np.sqrt(np.float32(D)))

F32 = mybir.dt.float32
BF16 = mybir.dt.bfloat16
AF = mybir.ActivationFunctionType
OP = mybir.AluOpType
AX = mybir.AxisListType


def build_nc():
    nc = bacc.Bacc(
        "TRN2",
        target_bir_lowering=False,
        debug=False,
        num_devices=NCORES,
    )
    x_d = nc.dram_tensor("x_shard", [RB, D], F32, kind="ExternalInput")
    al_d = nc.dram_tensor("alphas", [H, 3], F32, kind="ExternalInput")
    be_d = nc.dram_tensor("betas", [H, 3], F32, kind="ExternalInput")
    out_d = nc.dram_tensor("out_shard", [RB, D], F32, kind="ExternalOutput")

    with tile.TileContext(nc) as tc:
        with (
            tc.tile_pool(name="const", bufs=1) as cpool,
            tc.tile_pool(name="work", bufs=int(os.environ.get("KBENCH_WBUFS", "8"))) as wpool,
            tc.tile_pool(name="pair", bufs=int(os.environ.get("KBENCH_PBUFS", "3"))) as rpool,
            tc.tile_pool(name="junkp", bufs=2) as jpool,
            tc.tile_pool(name="small", bufs=6) as spool,
            tc.tile_pool(name="psum", bufs=1, space="PSUM") as ppool,
        ):
            # ---------------- prologue ----------------
            ones = cpool.tile([1, 128], F32, tag="ones")
            nc.vector.memset(ones[:], 1.0)

            x_flat = cpool.tile([1, RB * D], F32, tag="xflat")
            nc.sync.dma_start(
                x_flat[:], x_d.ap().rearrange("(o b) d -> o (b d)", o=1)
            )

            # alphas/betas as a single row [1, 48]: cols h*3+t, 24 + h*3+t
            ab_row = cpool.tile([1, 48], F32, tag="abrow")
            nc.sync.dma_start(
                ab_row[:, 0:24], al_d.ap().rearrange("(o h) t -> o (h t)", o=1)
            )
            nc.sync.dma_start(
                ab_row[:, 24:48], be_d.ap().rearrange("(o h) t -> o (h t)", o=1)
            )

            # broadcast scalars across partitions via PE outer product
            ab_psum = ppool.tile([128, 48], F32, tag="xb0")
            nc.tensor.matmul(ab_psum[:], ones[:], ab_row[:])
            AB = cpool.tile([128, 48], F32, tag="ab")
            nc.vector.tensor_copy(AB[:], ab_psum[:])

            # x in partition-major layout: X_part[p, b*NCH + c] = x[b, c*128+p]
            x_part = cpool.tile([128, RB * NCH], F32, tag="xpart")
            nc.sync.dma_start(
                x_part[:], x_d.ap().rearrange("b (c p) -> p (b c)", p=128)
            )

            # negak[:, h] = -a_k[h], bqk[:, h] = b_q[h] - b_k[h]
            ab3 = AB[:, 0:24].rearrange("p (h t) -> p h t", t=3)
            bb3 = AB[:, 24:48].rearrange("p (h t) -> p h t", t=3)
            negak = cpool.tile([128, H], F32, tag="negak")
            nc.vector.tensor_scalar_mul(negak[:], ab3[:, :, 1], -1.0)
            bqk = cpool.tile([128, H], F32, tag="bqk")
            nc.vector.tensor_sub(bqk[:], bb3[:, :, 0], bb3[:, :, 1])

            # KK[:, h*RB*NCH + b*NCH + c] = -(a_k x_i + b_k - b_q) for that column
            KK = cpool.tile([128, H * RB * NCH], F32, tag="kk")
            for h in range(H):
                nc.vector.tensor_scalar(
                    KK[:, h * RB * NCH : (h + 1) * RB * NCH],
                    x_part[:],
                    negak[:, h : h + 1],
                    bqk[:, h : h + 1],
                    op0=OP.mult,
                    op1=OP.add,
                )

            # X broadcast tiles: PSUM copy (for ACT) + SBUF copy (for DVE)
            xb_psum = []
            xb_sbuf = []
            for b in range(RB):
                pt = ppool.tile([128, D], F32, tag=f"xb{b}")
                nc.tensor.matmul(pt[:], ones[:], x_flat[:, b * D : (b + 1) * D])
                st = cpool.tile([128, D], F32, tag=f"xs{b}")
                nc.vector.tensor_copy(st[:], pt[:])
                xb_psum.append(pt)
                xb_sbuf.append(st)

            # stats: col = (b*NCH + c)*H + h
            NST = RB * NCH * H
            S_all = cpool.tile([128, NST], F32, tag="sall")
            SV_all = cpool.tile([128, NST], F32, tag="svall")

            # ---------------- main loop ----------------
            # Software-pipelined at the pair level: the front half (abs,
            # u, reciprocal) of pair p+1 is emitted before the back half
            # (exp, e*v, sum) of pair p, so no engine stalls on the
            # ACT->POOL->DVE round trip of the current pair.
            pair_state = {}
            sum_state = {}

            def emit_front(b, h):
                    # u_big: [u_c0 | u_c1 | u_c2 | u_c3 | umin4] so one FAST
                    # covers the reciprocals AND the stabilizer m = 1/umin
                    # (same approximation => top element cancels exactly).
                    u_big = rpool.tile([128, NCH * D + NCH], F32, tag="u")
                    for c in range(NCH):
                        kcol = h * RB * NCH + b * NCH + c
                        if "abs" in SKIP or "tsu" in SKIP:
                            continue
                        if c < ABS_DVE:
                            # DVE path: affine on 2x tensor_scalar, then
                            # max(|z|, eps) with fused row-min (== |z|+eps to
                            # fp32 noise for this eps).
                            z_t = wpool.tile([128, D], F32, tag="z")
                            nc.vector.tensor_scalar(
                                z_t[:],
                                xb_sbuf[b][:],
                                AB[:, 3 * h : 3 * h + 1],
                                KK[:, kcol : kcol + 1],
                                op0=OP.mult,
                                op1=OP.add,
                            )
                            nc.vector.tensor_scalar(
                                u_big[:, c * D : (c + 1) * D],
                                z_t[:],
                                EPS,
                                None,
                                op0=OP.abs_max,
                                op1=OP.min,
                                accum_out=u_big[:, NCH * D + c : NCH * D + c + 1],
                            )
                            continue
                        t_t = wpool.tile([128, D], F32, tag="t")
                        nc.scalar.activation(
                            t_t[:],
                            xb_psum[b][:],
                            AF.Abs,
                            bias=KK[:, kcol : kcol + 1],
                            scale=AB[:, 3 * h : 3 * h + 1],
                        )
                        nc.vector.tensor_scalar(
                            u_big[:, c * D : (c + 1) * D],
                            t_t[:],
                            EPS,
                            None,
                            op0=OP.add,
                            op1=OP.min,
                            accum_out=u_big[:, NCH * D + c : NCH * D + c + 1],
                        )
                    # r_big = -1/u_big; its umin tail is directly the exp
                    # bias -m (same approximation => exact top cancellation).
                    r_big = rpool.tile([128, NCH * D + NCH], F32, tag="r")
                    if "fast" in SKIP:
                        nc.vector.memset(r_big[:, 0:1], 0.0)
                        pair_state[(b, h)] = (r_big, r_big[:, NCH * D :], None)
                        return
                    c = dve_ops.RECIP_APPROX_FAST_CONSTS
                    nc.vector._custom_dve(
                        NEG_FAST_OP,
                        out=r_big[:],
                        in0=u_big[:],
                        s0=c["s0"],
                        s1=c["s1"],
                        imm2=c["imm2"],
                    )
                    negm4 = r_big[:, NCH * D : NCH * D + NCH]
                    v_t = None
                    if EV_GPSIMD and "sv" not in SKIP:
                        # v = a_v * x + b_v for this head (shared by 4 chunks)
                        v_t = rpool.tile([128, D], BF16 if BF16_EV else F32, tag="v")
                        v_eng = nc.gpsimd if V_GPSIMD else nc.vector
                        v_eng.tensor_scalar(
                            v_t[:],
                            xb_sbuf[b][:],
                            AB[:, 3 * h + 2 : 3 * h + 3],
                            AB[:, 24 + 3 * h + 2 : 24 + 3 * h + 3],
                            op0=OP.mult,
                            op1=OP.add,
                        )
                    pair_state[(b, h)] = (r_big, negm4, v_t)

            def emit_back(b, h):
                    r_big, negm4, v_t = pair_state.pop((b, h))
                    if EV_GPSIMD and MERGED_MULT:
                        # one big POOL multiply per pair (v repeated via a
                        # stride-0 AP); DVE still does 4 accumulating sums.
                        e_big = rpool.tile([128, NCH * D], F32, tag="ebig")
                        for c in range(NCH):
                            nc.scalar.activation(
                                e_big[:, c * D : (c + 1) * D],
                                r_big[:, c * D : (c + 1) * D],
                                AF.Exp,
                                scale=-1.0,
                                bias=negm4[:, c : c + 1],
                                accum_out=S_all[
                                    :, (b * NCH + c) * H + h : (b * NCH + c) * H + h + 1
                                ],
                            )
                        prod_big = rpool.tile([128, NCH * D], F32, tag="prodbig")
                        nc.gpsimd.tensor_tensor(
                            prod_big[:].rearrange("p (o d) -> p o d", d=D),
                            e_big[:].rearrange("p (o d) -> p o d", d=D),
                            v_t[:].rearrange("p (o d) -> p o d", o=1).broadcast_to(
                                [128, NCH, D]
                            ),
                            op=OP.mult,
                        )
                        for c in range(NCH):
                            scol = (b * NCH + c) * H + h
                            junk = jpool.tile([128, D], F32, tag="junk")
                            nc.vector.tensor_scalar(
                                junk[:],
                                prod_big[:, c * D : (c + 1) * D],
                                0.0,
                                None,
                                op0=OP.add,
                                op1=OP.add,
                                accum_out=SV_all[:, scol : scol + 1],
                            )
                        return
                    if EV_GPSIMD and RSUM:
                        # per-chunk POOL multiplies into one tile, then a
                        # single 3D min/add tensor_reduce for the 4 SV sums.
                        prod_big = rpool.tile([128, NCH * D], F32, tag="prodbig")
                        for c in range(NCH):
                            scol = (b * NCH + c) * H + h
                            e_t = wpool.tile([128, D], F32, tag="e")
                            nc.scalar.activation(
                                e_t[:],
                                r_big[:, c * D : (c + 1) * D],
                                AF.Exp,
                                scale=-1.0,
                                bias=negm4[:, c : c + 1],
                                accum_out=S_all[:, scol : scol + 1],
                            )
                            nc.gpsimd.tensor_tensor(
                                prod_big[:, c * D : (c + 1) * D],
                                e_t[:],
                                v_t[:],
                                op=OP.mult,
                            )
                        sv_view = SV_all[:].rearrange("p (g hh) -> p g hh", hh=H)[
                            :, b * NCH : (b + 1) * NCH, h
                        ]
                        nc.vector.tensor_reduce(
                            sv_view,
                            prod_big[:].rearrange("p (g dd) -> p g dd", dd=D),
                            axis=AX.X,
                            op=OP.add,
                        )
                        return
                    for c in range(NCH):
                        scol = (b * NCH + c) * H + h
                        if "exp" in SKIP:
                            continue
                        e_t = wpool.tile([128, D], BF16 if BF16_EV else F32, tag="e")
                        nc.scalar.activation(
                            e_t[:],
                            r_big[:, c * D : (c + 1) * D],
                            AF.Exp,
                            scale=-1.0,
                            bias=negm4[:, c : c + 1],
                            accum_out=S_all[:, scol : scol + 1],
                        )
                        if "sv" in SKIP:
                            continue
                        gidx = (b * H + h) * NCH + c
                        if EV_GPSIMD and AMR_EVERY and gidx % AMR_EVERY == 0:
                            # striped load-balancing: this chunk's e*v runs as
                            # one fused DVE op instead of POOL mult + DVE sum.
                            junk = jpool.tile([128, D], F32, tag="junk")
                            nc.vector.affine_mul_reduce(
                                out=junk[:],
                                accum_out=SV_all[:, scol : scol + 1],
                                in0=xb_sbuf[b][:],
                                in1=e_t[:],
                                scale=AB[:, 3 * h + 2 : 3 * h + 3],
                                bias=AB[:, 24 + 3 * h + 2 : 24 + 3 * h + 3],
                            )
                        elif EV_GPSIMD:
                            # GPSIMD multiplies, DVE sums at 2x mode: cheaper
                            # on DVE than one fused 1x affine_mul_reduce.
                            prod = wpool.tile([128, D], BF16 if BF16_EV else F32, tag="prod")
                            nc.gpsimd.tensor_tensor(
                                prod[:], e_t[:], v_t[:], op=OP.mult
                            )
                            sum_state.setdefault((b, h), []).append((prod, scol))
                        elif False:
                            pass
                        else:
                            junk = jpool.tile([128, D], F32, tag="junk")
                            nc.vector.affine_mul_reduce(
                                out=junk[:],
                                accum_out=SV_all[:, scol : scol + 1],
                                in0=xb_sbuf[b][:],
                                in1=e_t[:],
                                scale=AB[:, 3 * h + 2 : 3 * h + 3],
                                bias=AB[:, 24 + 3 * h + 2 : 24 + 3 * h + 3],
                            )

            def emit_sums(b, h):
                    for prod, scol in sum_state.pop((b, h), []):
                        junk = jpool.tile([128, D], BF16 if BF16_EV else F32, tag="junk")
                        nc.vector.tensor_scalar(
                            junk[:],
                            prod[:],
                            0.0,
                            None,
                            op0=OP.add,
                            op1=OP.add,
                            accum_out=SV_all[:, scol : scol + 1],
                        )

            pairs = [(b, h) for b in range(RB) for h in range(H)]
            LOOKAHEAD = int(os.environ.get("KBENCH_LOOKAHEAD", "2"))
            LA_SUM = LOOKAHEAD + int(os.environ.get("KBENCH_LA_SUM", "0"))

            def emit_main():
                for i in range(len(pairs) + LA_SUM):
                    if i < len(pairs):
                        emit_front(*pairs[i])
                    if i >= LOOKAHEAD and i - LOOKAHEAD < len(pairs):
                        emit_back(*pairs[i - LOOKAHEAD])
                    if i >= LA_SUM:
                        emit_sums(*pairs[i - LA_SUM])

            # bench-only: repeat the (idempotent) main loop on-device so the
            # kernel time can be read off a wall-clock slope over repeats.
            repeat = int(os.environ.get("KBENCH_REPEAT", "1"))
            if repeat > 1:
                with tc.For_i(0, repeat, 1):
                    emit_main()
            else:
                emit_main()

            # ---------------- epilogue ----------------
            if SKIP:
                # timing-only mode: bypass stats (some were never written)
                nc.sync.dma_start(
                    out_d.ap().rearrange("b (c p) -> p b c", p=128),
                    x_part[:].rearrange("p (b c) -> p b c", c=NCH),
                )
            else:
                s_inv = cpool.tile([128, NST], F32, tag="sinv")
                nc.vector.reciprocal(s_inv[:], S_all[:])
                z = cpool.tile([128, NST], F32, tag="z")
                nc.vector.tensor_mul(z[:], SV_all[:], s_inv[:])
                att = cpool.tile([128, RB * NCH], F32, tag="att")
                nc.vector.tensor_reduce(
                    att[:],
                    z[:].rearrange("p (g h) -> p g h", h=H),
                    axis=AX.X,
                    op=OP.add,
                )
                out_c = cpool.tile([128, RB * NCH], F32, tag="outc")
                nc.vector.scalar_tensor_tensor(
                    out=out_c[:],
                    in0=att[:],
                    scalar=INV_SQRT_D,
                    in1=x_part[:],
                    op0=OP.mult,
                    op1=OP.add,
                )
                nc.sync.dma_start(
                    out_d.ap().rearrange("b (c p) -> p b c", p=128),
                    out_c[:].rearrange("p (b c) -> p b c", c=NCH),
                )

    nc.compile()
    return nc


_NC_CACHE = None


def get_nc():
    global _NC_CACHE
    if _NC_CACHE is None:
        _NC_CACHE = build_nc()
    return _NC_CACHE


def kernel(x: np.ndarray, alphas: np.ndarray, betas: np.ndarray) -> np.ndarray:
    x = np.ascontiguousarray(x, dtype=np.float32)
    alphas = np.ascontiguousarray(alphas, dtype=np.float32)
    betas = np.ascontiguousarray(betas, dtype=np.float32)

    nc = get_nc()
    in_maps = [
        {
            "x_shard": x[c * RB : (c + 1) * RB],
            "alphas": alphas,
            "betas": betas,
        }
        for c in range(NCORES)
    ]
    res = run_bass_kernel_spmd(nc, in_maps, core_ids=list(range(NCORES)))
    out = np.concatenate([res.results[c]["out_shard"] for c in range(NCORES)], axis=0)
    return out


if __name__ == "__main__":
    rng = np.random.default_rng(0)
    x = rng.standard_normal((B, D), dtype=np.float32)
    al = rng.random((H, 3), dtype=np.float32)
    be = rng.random((H, 3), dtype=np.float32)
    out = kernel(x=x, alphas=al, betas=be)
    print("out", out.shape, out.dtype, float(np.abs(out).max()))

